# revision 1
# baseline (speedup 1.0000x reference)
"""Trainium2 Bass kernel for nn_Contour_to_mask (winding-number soft
rasterization of a 128-point contour into a (1, 2, 256, 256) f32 mask).

Math: for pixel m = (mx, my) = (i/256, j/256) and edge (c_n, c_{n+1}):
  cross_n(m) = (cy*cxn - cx*cyn) + (cyn-cy)*mx + (cx-cxn)*my
  dot_n(m)   = (cx*cxn + cy*cyn) - (cx+cxn)*mx - (cy+cyn)*my + mx^2 + my^2
Both are SEPARABLE into per-edge row/column profiles:
  cross[n, i, j] = Pc[n, i] + Qc[n, j];   dot[n, i, j] = Rd[n, i] + Sd[n, j].
  angle = arccos(clip(cos, -1+eps, 1-eps)) == pi/2 - arctan(clip(r, +-R1))
  with r = dot/|cross|, R1 = cot(arccos(1-eps)).
  contribution = tanh(1e5*cross)*angle; winding = |sum_n contrib|/2pi, clip.

Engine split per 2048-pixel (8-image-row) superblock (partitions = 128 edges):
  ACT:  4 cross row-builds (Identity w/ per-partition bias), s = tanh(1e5*
        cross) -> bf16, phi = arctan(rc). Tanh+Arctan share one table set.
  GPSIMD: 4 cross row-builds (tensor_scalar add), t2 = s*phi.
  DVE:  v = reciprocal_approx_fast(cross); per-row custom fused op
        rc = clip((Sd + Rd_i)*|v|, +-R1) that BUILDS dot inline (Sd tensor +
        per-partition scalar Rd_i) and guards NaN via select(m==m)
        (cross==+-0 -> v=NaN -> rc:=R1; s=tanh(0)=0 kills it, matching ref).
  PE:   reduction over the 128 edges via sliding-window one-hot lhsT matmuls
        into two PSUM tiles: accS = sum(s) (bf16 rhs, full-rate) and
        accT = sum(t2) (fp32 rhs); finale w = min(|pi/2*accS - accT|/2pi, 1).

Sharding: 8 cores; core c handles batch c//4, image rows [(c%4)*64, +64).
"""
import sys

sys.path.insert(0, "/opt/trn_rl_repo")

import numpy as np

SIZE = 256
K_TANH = 100000.0
EPS = 1e-5
B = 2
NPTS = 128
N_CORES = 8
PIX = SIZE * SIZE              # 65536
PIX_CORE = PIX * B // N_CORES  # 16384 pixels per core
ROWS_CORE = PIX_CORE // SIZE   # 64 image rows per core
BLK = 512                      # pixels per reduction block (one PSUM bank)
NBLK = PIX_CORE // BLK         # 32
SBLK = 2048                    # pixels per elementwise superblock (8 rows)
NSBLK = PIX_CORE // SBLK       # 8

_compiled = {}
_ops = {}


def _clip_bound():
    c = np.float64(np.float32(1.0 - EPS))
    return np.float32(c / np.sqrt(1.0 - c * c))


def _register_ops():
    """Register the two custom DVE ops (idempotent)."""
    if _ops:
        return _ops
    from concourse import dve_ops
    from concourse.dve_spec import (
        Spec, Src0, Src1, C0, C1, C2, Zero, maxx, minn, eq, select, lower)
    from concourse.dve_uop import DveOpSpec

    def reg(name, spec):
        if name in dve_ops._SUB_OPCODE_FOR_NAME:
            return next(op for op in dve_ops.OPS if op.name == name)
        row = dve_ops._CUSTOM_DVE_ROW_BASE + len(dve_ops.OPS)
        sha = {ver: DveOpSpec(name=name, opcode=row,
                              uops=lower(spec, ver=ver), rd1_en=True).sha(ver)
               for ver in ("v3", "v4")}
        op = dve_ops.DveOp(name, spec, subdim=False, uops_sha=sha)
        dve_ops.OPS.append(op)
        dve_ops.CUSTOM_DVE_SPECS[name] = spec
        dve_ops._SUB_OPCODE_FOR_NAME[name] = row
        return op

    # DOT_RMUL_CLIP: out = clip((Src1 + C0)*|Src0|, -C1, C1); NaN -> C1.
    # Src0 = v (recip of cross), Src1 = Sd column profile, C0 = Rd_i scalar.
    _d = Src1 + C0
    _av = maxx(Src0, Zero - Src0)
    _m = _d * _av
    _cl = minn(maxx(_m, Zero - C1), C1)
    _body = select(eq(_m, _m), _cl, C1)

    def _ref_rmul(in0, in1, s0, s1, imm2):
        m = (in1 + s0) * np.abs(in0)
        out = np.minimum(np.maximum(m, -s1), s1)
        return np.where(np.isnan(m), s1, out).astype(np.float32)

    _ops["rmul"] = reg("DOT_RMUL_CLIP", Spec(body=_body, reference=_ref_rmul))

    # FINALE: out = min(|Src0*C0 - Src1| * C1, C2)
    _fd = Src0 * C0 - Src1
    _fa = maxx(_fd, Zero - _fd)
    _fbody = minn(_fa * C1, C2)

    def _ref_fin(in0, in1, s0, s1, imm2):
        return np.minimum(np.abs(in0 * s0 - in1) * s1, imm2).astype(np.float32)

    _ops["fin"] = reg("WINDING_FINALE", Spec(body=_fbody, reference=_ref_fin))
    return _ops


def _build(repeat=1):
    import concourse.bacc as bacc
    import concourse.tile as tile
    import concourse.mybir as mybir

    AF = mybir.ActivationFunctionType
    ALU = mybir.AluOpType
    f32 = mybir.dt.float32
    bf16 = mybir.dt.bfloat16
    ops = _register_ops()

    nc = bacc.Bacc("TRN2", target_bir_lowering=False, debug=False,
                   num_devices=N_CORES)

    pc_d = nc.dram_tensor("pc", [NPTS, ROWS_CORE], f32, kind="ExternalInput").ap()
    qc_d = nc.dram_tensor("qc", [NPTS, SIZE], f32, kind="ExternalInput").ap()
    rd_d = nc.dram_tensor("rd", [NPTS, ROWS_CORE], f32, kind="ExternalInput").ap()
    sd_d = nc.dram_tensor("sd", [NPTS, SIZE], f32, kind="ExternalInput").ap()
    redp_d = nc.dram_tensor("redp", [NPTS, 63], bf16, kind="ExternalInput").ap()
    redm_d = nc.dram_tensor("redm", [NPTS, 63], f32, kind="ExternalInput").ap()
    out_d = nc.dram_tensor("out", [NBLK, BLK], f32, kind="ExternalOutput").ap()

    R1 = float(_clip_bound())
    RPB = SBLK // SIZE  # rows per superblock = 8
    BPB = SBLK // BLK   # reduction blocks per superblock = 4

    with tile.TileContext(nc) as tc:
        with tc.tile_pool(name="cst", bufs=1) as cst, \
             tc.tile_pool(name="work", bufs=3) as work, \
             tc.tile_pool(name="pacc", bufs=1, space="PSUM") as pacc:
            pc_t = cst.tile([NPTS, ROWS_CORE], f32, name="pc_t")
            qc_t = cst.tile([NPTS, SIZE], f32, name="qc_t")
            rd_t = cst.tile([NPTS, ROWS_CORE], f32, name="rd_t")
            sd_t = cst.tile([NPTS, SIZE], f32, name="sd_t")
            redp_t = cst.tile([NPTS, 63], bf16, name="redp_t")
            redm_t = cst.tile([NPTS, 63], f32, name="redm_t")
            nc.sync.dma_start(pc_t[:], pc_d[:])
            nc.sync.dma_start(qc_t[:], qc_d[:])
            nc.sync.dma_start(rd_t[:], rd_d[:])
            nc.sync.dma_start(sd_t[:], sd_d[:])
            nc.sync.dma_start(redp_t[:], redp_d[:])
            nc.sync.dma_start(redm_t[:], redm_d[:])

            accS = pacc.tile([NBLK, BLK], f32, name="accS")
            accT = pacc.tile([NBLK, BLK], f32, name="accT")

            for rep in range(repeat):
                for u in range(NSBLK):
                    cross = work.tile([NPTS, SBLK], f32, tag="cross",
                                      name=f"cross{rep}_{u}")
                    for h in range(RPB):
                        i = u * RPB + h  # local image row
                        hs = slice(h * SIZE, (h + 1) * SIZE)
                        if h % 2 == 1:
                            nc.gpsimd.tensor_scalar(
                                cross[:, hs], qc_t[:], pc_t[:, i:i + 1], None,
                                ALU.add)
                        else:
                            nc.vector.tensor_scalar(
                                cross[:, hs], qc_t[:], pc_t[:, i:i + 1], None,
                                ALU.add)

                    s = work.tile([NPTS, SBLK], bf16, tag="s", name=f"s{rep}_{u}")
                    for g in range(2):
                        gs = slice(g * (SBLK // 2), (g + 1) * (SBLK // 2))
                        nc.scalar.activation(s[:, gs], cross[:, gs], AF.Tanh,
                                             scale=K_TANH)

                    v = work.tile([NPTS, SBLK], f32, tag="v", name=f"v{rep}_{u}")
                    nc.vector.reciprocal_approx_fast(v[:], cross[:])

                    rc = work.tile([NPTS, SBLK], f32, tag="rc", name=f"rc{rep}_{u}")
                    for h in range(RPB):
                        i = u * RPB + h
                        hs = slice(h * SIZE, (h + 1) * SIZE)
                        nc.vector._custom_dve(
                            ops["rmul"], out=rc[:, hs], in0=v[:, hs],
                            in1=sd_t[:], s0=rd_t[:, i:i + 1], s1=R1)

                    phi = work.tile([NPTS, SBLK], f32, tag="phi",
                                    name=f"phi{rep}_{u}")
                    for g in range(2):
                        gs = slice(g * (SBLK // 2), (g + 1) * (SBLK // 2))
                        nc.scalar.activation(phi[:, gs], rc[:, gs], AF.Arctan)

                    t2 = work.tile([NPTS, SBLK], f32, tag="t2", name=f"t2{rep}_{u}")
                    for g in range(4):
                        gs = slice(g * (SBLK // 4), (g + 1) * (SBLK // 4))
                        nc.gpsimd.tensor_tensor(t2[:, gs], s[:, gs], phi[:, gs],
                                                ALU.mult)

                    for h in range(BPB):
                        j = BPB * u + h
                        hs = slice(h * BLK, (h + 1) * BLK)
                        lp = redp_t[:, 31 - j:63 - j]
                        lm = redm_t[:, 31 - j:63 - j]
                        nc.tensor.matmul(accS[:], lp, s[:, hs],
                                         start=(j == 0), stop=False)
                        nc.tensor.matmul(accT[:], lm, t2[:, hs],
                                         start=(j == 0),
                                         stop=(j == NBLK - 1 and
                                               rep == repeat - 1))

            tcopy = work.tile([NBLK, BLK], f32, tag="tcopy", name="tcopy")
            nc.vector.tensor_copy(tcopy[:], accT[:])
            w = work.tile([NBLK, BLK], f32, tag="w", name="w")
            nc.vector._custom_dve(
                ops["fin"], out=w[:], in0=accS[:], in1=tcopy[:],
                s0=float(np.float32(np.pi / 2)),
                s1=float(np.float32(1.0 / (2.0 * np.pi))), imm2=1.0)
            nc.sync.dma_start(out_d[:], w[:])

    nc.compile()
    return nc


def _host_inputs(contour: np.ndarray):
    """Per-core in_maps from the full (B, NPTS, 2) contour."""
    mx = (np.arange(SIZE) / SIZE).astype(np.float64)   # i profile
    my = (np.arange(SIZE) / SIZE).astype(np.float64)   # j profile

    prof = []
    for b in range(B):
        cx = contour[b, :, 0].astype(np.float64)
        cy = contour[b, :, 1].astype(np.float64)
        cxn = np.roll(cx, -1)
        cyn = np.roll(cy, -1)
        A = cy * cxn - cx * cyn
        Bc = cyn - cy
        Cc = cx - cxn
        Dd = cx * cxn + cy * cyn
        Ed = -(cx + cxn)
        Fd = -(cy + cyn)
        Pc = (A[:, None] + Bc[:, None] * mx[None, :]).astype(np.float32)
        Qc = (Cc[:, None] * my[None, :]).astype(np.float32)
        Rd = (Dd[:, None] + Ed[:, None] * mx[None, :] + mx[None, :] ** 2
              ).astype(np.float32)
        Sd = (Fd[:, None] * my[None, :] + my[None, :] ** 2).astype(np.float32)
        prof.append((Pc, Qc, Rd, Sd))

    import ml_dtypes
    redp = np.zeros((NPTS, 63), dtype=ml_dtypes.bfloat16)
    redp[:, 31] = 1.0
    redm = np.zeros((NPTS, 63), dtype=np.float32)
    redm[:, 31] = 1.0

    in_maps = []
    for c in range(N_CORES):
        b = c // (N_CORES // B)
        r0 = (c % (N_CORES // B)) * ROWS_CORE
        Pc, Qc, Rd, Sd = prof[b]
        in_maps.append({
            "pc": np.ascontiguousarray(Pc[:, r0:r0 + ROWS_CORE]),
            "qc": Qc,
            "rd": np.ascontiguousarray(Rd[:, r0:r0 + ROWS_CORE]),
            "sd": Sd,
            "redp": redp,
            "redm": redm,
        })
    return in_maps


def kernel(contour: np.ndarray) -> np.ndarray:
    from concourse import bass_utils

    contour = np.asarray(contour, dtype=np.float32)
    if "nc" not in _compiled:
        _compiled["nc"] = _build()
    in_maps = _host_inputs(contour)
    res = bass_utils.run_bass_kernel_spmd(
        _compiled["nc"], in_maps, core_ids=list(range(N_CORES))).results

    mask = np.zeros((1, B, SIZE, SIZE), dtype=np.float32)
    for c in range(N_CORES):
        b = c // (N_CORES // B)
        r0 = (c % (N_CORES // B)) * ROWS_CORE
        mask[0, b, r0:r0 + ROWS_CORE, :] = (
            res[c]["out"].reshape(ROWS_CORE, SIZE))
    return mask



# revision 3
# speedup vs baseline: 1.7378x; 1.7378x over previous
"""Trainium2 Bass kernel for nn_Contour_to_mask (winding-number soft
rasterization of a 128-point contour into a (1, 2, 256, 256) f32 mask).

v8 "three-pass" design. Math: for pixel (i,j) and edge n,
  cross = Pc[n,i] + Qc[n,j],  dot = Rd[n,i] + Sd[n,j]   (separable profiles)
  contribution = s*(pi/2 - arctan(dot/|cross|)) with s = tanh(1e5*cross)
Approximated (rel-err ~1.4e-2 < 2e-2 gate, verified vs reference) as
  contribution = (pi/2)*sgn(cross) - arctan(dot/cross)
so the per-element work collapses to THREE elementwise passes:
  DVE :  one fused 7-stage custom op per image row:
           x = Qc + Pc_i                 (cross, built inline)
           z = BITWISE_NOT(x) * c0'      (fast-reciprocal seed)
           m = z*(1 - x*z)               (1 Newton step; ~0.22% rel err
                                          with c1^2 pre-folded into Sd/Rd)
           q = (Sd'' + Rd''_i) * m       (signed dot/cross, unclipped)
  ACT :  phi = Arctan(q) in f32 (one big call per superblock), plus a
         share of the sign pass as Sign(Qc*1 + Pc_i) (inline bias).
  Pool:  rest of the sign pass as tensor_scalar is_gt: g = (Qc > -Pc_i).
  PE  :  two fp32r one-hot sliding-window reductions over the 128 edges
         into PSUM rows (full-rate at 512-wide blocks): accT = sum(T),
         accP = sum(phi).
  finale (DVE custom): w = min(|accT*C0 + C1 - accP| / 2pi, 1) with
         per-partition C0/C1 selecting the row-pair's sign encoding
         (Pool ge rows: C0=pi, C1=-64pi; ACT Sign rows: C0=pi/2, C1=0).
Host side pre-scales Sd/Rd by c1^2, and nudges Pc by 1 ulp wherever
f32(Qc+Pc) would be exactly/nearly zero (kills the NaN path of the
BITWISE_NOT seed; ~0-2 pixels per run, winding effect < 1e-7).

Sharding: 8 cores; core c handles batch c//4, image rows [(c%4)*64, +64).
"""
import sys

sys.path.insert(0, "/opt/trn_rl_repo")

import numpy as np

SIZE = 256
B = 2
NPTS = 128
N_CORES = 8
ROWS_CORE = SIZE * SIZE * B // (N_CORES * SIZE)  # 64 image rows per core
RPB = 8                        # rows per superblock
SBLK = RPB * SIZE              # 2048 px
NSBLK = ROWS_CORE // RPB       # 8
BLK = 512                      # pixels per reduction block (one PSUM bank)
NBLK = ROWS_CORE * SIZE // BLK  # 32
ROWS_PER_BLK = BLK // SIZE     # 2 image rows per PSUM row

# tuned on a +-[1,2) mantissa grid: max rel err 0.224% for the 1-NR
# BITWISE_NOT reciprocal z*(1-x*z) with output scale C1SQ folded into Sd/Rd
C0P = -0.11853305
C1SQ = 4.00896949

# row-pair sign-pass flavor: True -> ACT Sign, False -> Pool is_gt.
# 1 ACT pair per superblock (4 pairs) balances ACT vs Pool busy time.
ACT_PAIR = [(p % 4 == 3) for p in range(NBLK)]

_compiled = {}
_ops = {}


def _register_ops():
    if _ops:
        return _ops
    from concourse import dve_ops
    from concourse.dve_spec import (
        Spec, Src0, Src1, C0, C1, C2, One, minn, lower, AluOp, Bin)
    from concourse.dve_uop import DveOpSpec

    def reg(name, spec):
        if name in dve_ops._SUB_OPCODE_FOR_NAME:
            return next(op for op in dve_ops.OPS if op.name == name)
        row = dve_ops._CUSTOM_DVE_ROW_BASE + len(dve_ops.OPS)
        sha = {ver: DveOpSpec(name=name, opcode=row,
                              uops=lower(spec, ver=ver), rd1_en=True).sha(ver)
               for ver in ("v3", "v4")}
        op = dve_ops.DveOp(name, spec, subdim=False, uops_sha=sha)
        dve_ops.OPS.append(op)
        dve_ops.CUSTOM_DVE_SPECS[name] = spec
        dve_ops._SUB_OPCODE_FOR_NAME[name] = row
        return op

    f32 = np.float32

    # FUSEDQ: q = (Src1 + C1) * (z*(1-x*z)), x = Src0 + C0, z = NOT(x)*C2
    _x = Src0 + C0
    _z = Bin(AluOp.BITWISE_NOT, _x, _x) * C2
    _m = _z * (One - _x * _z)
    _body_q = (Src1 + C1) * _m

    def _ref_q(in0, in1, s0, s1, imm2):
        x = (in0 + s0).astype(f32)
        z = ((~x.view(np.int32)).view(f32) * f32(imm2)).astype(f32)
        m = (z * (f32(1.0) - (x * z).astype(f32)).astype(f32)).astype(f32)
        return ((in1 + s1).astype(f32) * m).astype(f32)

    _ops["q"] = reg("FUSEDQ_V8", Spec(body=_body_q, reference=_ref_q))

    # FIN2: w = min(|Src0*C0 + C1 - Src1| * C2, 1)
    _b = Src0 * C0 + C1
    _body_f = minn(Bin(AluOp.ABSOLUTE_DIFF, _b, Src1) * C2, One)

    def _ref_f(in0, in1, s0, s1, imm2):
        return np.minimum(
            np.abs((in0 * s0 + s1) - in1) * f32(imm2), f32(1.0)).astype(f32)

    _ops["fin"] = reg("FIN2_V8", Spec(body=_body_f, reference=_ref_f))
    return _ops


def _build():
    import concourse.bacc as bacc
    import concourse.tile as tile
    import concourse.mybir as mybir

    AF = mybir.ActivationFunctionType
    ALU = mybir.AluOpType
    f32 = mybir.dt.float32
    f32r = mybir.dt.float32r
    ops = _register_ops()

    nc = bacc.Bacc("TRN2", target_bir_lowering=False, debug=False,
                   num_devices=N_CORES)

    qc_d = nc.dram_tensor("qc", [NPTS, SIZE], f32, kind="ExternalInput").ap()
    sd2_d = nc.dram_tensor("sd2", [NPTS, SIZE], f32, kind="ExternalInput").ap()
    pc_d = nc.dram_tensor("pc", [NPTS, ROWS_CORE], f32, kind="ExternalInput").ap()
    npc_d = nc.dram_tensor("npc", [NPTS, ROWS_CORE], f32, kind="ExternalInput").ap()
    rd2_d = nc.dram_tensor("rd2", [NPTS, ROWS_CORE], f32, kind="ExternalInput").ap()
    redg_d = nc.dram_tensor("redg", [NPTS, 63], f32r, kind="ExternalInput").ap()
    fc0_d = nc.dram_tensor("fc0", [NBLK, 1], f32, kind="ExternalInput").ap()
    fc1_d = nc.dram_tensor("fc1", [NBLK, 1], f32, kind="ExternalInput").ap()
    out_d = nc.dram_tensor("out", [NBLK, BLK], f32, kind="ExternalOutput").ap()

    with tile.TileContext(nc) as tc:
        with tc.tile_pool(name="cst", bufs=1) as cst, \
             tc.tile_pool(name="work", bufs=3) as work, \
             tc.tile_pool(name="pacc", bufs=1, space="PSUM") as pacc:
            qc_t = cst.tile([NPTS, SIZE], f32, name="qc_t")
            sd2_t = cst.tile([NPTS, SIZE], f32, name="sd2_t")
            pc_t = cst.tile([NPTS, ROWS_CORE], f32, name="pc_t")
            npc_t = cst.tile([NPTS, ROWS_CORE], f32, name="npc_t")
            rd2_t = cst.tile([NPTS, ROWS_CORE], f32, name="rd2_t")
            redg_t = cst.tile([NPTS, 63], f32r, name="redg_t")
            fc0_t = cst.tile([NBLK, 1], f32, name="fc0_t")
            fc1_t = cst.tile([NBLK, 1], f32, name="fc1_t")
            nc.sync.dma_start(qc_t[:], qc_d[:])
            nc.sync.dma_start(sd2_t[:], sd2_d[:])
            nc.sync.dma_start(pc_t[:], pc_d[:])
            nc.sync.dma_start(npc_t[:], npc_d[:])
            nc.sync.dma_start(rd2_t[:], rd2_d[:])
            nc.sync.dma_start(redg_t[:], redg_d[:])
            nc.sync.dma_start(fc0_t[:], fc0_d[:])
            nc.sync.dma_start(fc1_t[:], fc1_d[:])

            accT = pacc.tile([NBLK, BLK], f32, name="accT")
            accP = pacc.tile([NBLK, BLK], f32, name="accP")

            for u in range(NSBLK):
                q = work.tile([NPTS, SBLK], f32, tag="q", name=f"q{u}")
                for h in range(RPB):
                    i = u * RPB + h
                    hs = slice(h * SIZE, (h + 1) * SIZE)
                    nc.vector._custom_dve(
                        ops["q"], out=q[:, hs], in0=qc_t[:], in1=sd2_t[:],
                        s0=pc_t[:, i:i + 1], s1=rd2_t[:, i:i + 1], imm2=C0P)

                g = work.tile([NPTS, SBLK], f32r, tag="g", name=f"g{u}")
                for h in range(RPB):
                    i = u * RPB + h
                    hs = slice(h * SIZE, (h + 1) * SIZE)
                    if ACT_PAIR[i // ROWS_PER_BLK]:
                        nc.scalar.activation(g[:, hs], qc_t[:], AF.Sign,
                                             bias=pc_t[:, i:i + 1])
                    else:
                        nc.gpsimd.tensor_scalar(g[:, hs], qc_t[:],
                                                npc_t[:, i:i + 1], None,
                                                ALU.is_gt)

                phi = work.tile([NPTS, SBLK], f32r, tag="phi", name=f"phi{u}")
                nc.scalar.activation(phi[:], q[:], AF.Arctan)

                for h2 in range(SBLK // BLK):
                    j = (SBLK // BLK) * u + h2
                    hs = slice(h2 * BLK, (h2 + 1) * BLK)
                    lw = redg_t[:, 31 - j:63 - j]
                    nc.tensor.matmul(accT[:], lw, g[:, hs],
                                     start=(j == 0), stop=(j == NBLK - 1))
                    nc.tensor.matmul(accP[:], lw, phi[:, hs],
                                     start=(j == 0), stop=(j == NBLK - 1))

            pcopy = work.tile([NBLK, BLK], f32, tag="pcopy", name="pcopy")
            nc.scalar.copy(pcopy[:], accP[:])
            w = work.tile([NBLK, BLK], f32, tag="w", name="w")
            nc.vector._custom_dve(
                ops["fin"], out=w[:], in0=accT[:], in1=pcopy[:],
                s0=fc0_t[:], s1=fc1_t[:],
                imm2=float(np.float32(1.0 / (2.0 * np.pi))))
            nc.sync.dma_start(out_d[:], w[:])

    nc.compile()
    return nc


def _host_inputs(contour: np.ndarray):
    """Per-core in_maps from the full (B, NPTS, 2) contour."""
    f32 = np.float32
    mx = (np.arange(SIZE) / SIZE).astype(np.float64)
    my = mx

    prof = []
    for b in range(B):
        cx = contour[b, :, 0].astype(np.float64)
        cy = contour[b, :, 1].astype(np.float64)
        cxn = np.roll(cx, -1)
        cyn = np.roll(cy, -1)
        A = cy * cxn - cx * cyn
        Bc = cyn - cy
        Cc = cx - cxn
        Dd = cx * cxn + cy * cyn
        Ed = -(cx + cxn)
        Fd = -(cy + cyn)
        Pc = (A[:, None] + Bc[:, None] * mx[None, :]).astype(f32)
        Qc = (Cc[:, None] * my[None, :]).astype(f32)
        Rd2 = ((Dd[:, None] + Ed[:, None] * mx[None, :] + mx[None, :] ** 2)
               * C1SQ).astype(f32)
        Sd2 = ((Fd[:, None] * my[None, :] + my[None, :] ** 2) * C1SQ).astype(f32)
        # zero-exterminator: f32(Qc + Pc_i) == +-0 would NaN the NOT seed.
        for _ in range(4):
            cross = Qc[:, None, :] + Pc[:, :, None]  # f32 [N, i, j]
            n_, i_ = np.nonzero((np.abs(cross) < 1e-30).any(axis=2))
            if len(n_) == 0:
                break
            Pc[n_, i_] = np.nextafter(Pc[n_, i_], f32(np.inf), dtype=f32)
        prof.append((Pc, Qc, Rd2, Sd2))

    redg = np.zeros((NPTS, 63), dtype=f32)
    redg[:, 31] = 1.0
    fc0 = np.where(ACT_PAIR, f32(np.pi / 2), f32(np.pi)).astype(f32)[:, None]
    fc1 = np.where(ACT_PAIR, f32(0.0), f32(-64.0 * np.pi)).astype(f32)[:, None]

    in_maps = []
    for c in range(N_CORES):
        b = c // (N_CORES // B)
        r0 = (c % (N_CORES // B)) * ROWS_CORE
        Pc, Qc, Rd2, Sd2 = prof[b]
        pc = np.ascontiguousarray(Pc[:, r0:r0 + ROWS_CORE])
        in_maps.append({
            "qc": Qc,
            "sd2": Sd2,
            "pc": pc,
            "npc": -pc,
            "rd2": np.ascontiguousarray(Rd2[:, r0:r0 + ROWS_CORE]),
            "redg": redg,
            "fc0": fc0,
            "fc1": fc1,
        })
    return in_maps


def kernel(contour: np.ndarray) -> np.ndarray:
    from concourse import bass_utils

    contour = np.asarray(contour, dtype=np.float32)
    if "nc" not in _compiled:
        _compiled["nc"] = _build()
    in_maps = _host_inputs(contour)
    res = bass_utils.run_bass_kernel_spmd(
        _compiled["nc"], in_maps, core_ids=list(range(N_CORES))).results

    mask = np.zeros((1, B, SIZE, SIZE), dtype=np.float32)
    for c in range(N_CORES):
        b = c // (N_CORES // B)
        r0 = (c % (N_CORES // B)) * ROWS_CORE
        mask[0, b, r0:r0 + ROWS_CORE, :] = (
            res[c]["out"].reshape(ROWS_CORE, SIZE))
    return mask


# revision 5
# speedup vs baseline: 1.7457x; 1.0046x over previous
"""Trainium2 Bass kernel for nn_Contour_to_mask (winding-number soft
rasterization of a 128-point contour into a (1, 2, 256, 256) f32 mask).

v8 "three-pass" design. Math: for pixel (i,j) and edge n,
  cross = Pc[n,i] + Qc[n,j],  dot = Rd[n,i] + Sd[n,j]   (separable profiles)
  contribution = s*(pi/2 - arctan(dot/|cross|)) with s = tanh(1e5*cross)
Approximated (rel-err ~1.4e-2 < 2e-2 gate, verified vs reference) as
  contribution = (pi/2)*sgn(cross) - arctan(dot/cross)
so the per-element work collapses to THREE elementwise passes:
  DVE :  one fused 7-stage custom op per image row:
           x = Qc + Pc_i                 (cross, built inline)
           z = BITWISE_NOT(x) * c0'      (fast-reciprocal seed)
           m = z*(1 - x*z)               (1 Newton step; ~0.22% rel err
                                          with c1^2 pre-folded into Sd/Rd)
           q = (Sd'' + Rd''_i) * m       (signed dot/cross, unclipped)
  ACT :  phi = Arctan(q) in f32 (one big call per superblock), plus a
         share of the sign pass as Sign(Qc*1 + Pc_i) (inline bias).
  Pool:  rest of the sign pass as tensor_scalar is_gt: g = (Qc > -Pc_i).
  PE  :  two fp32r one-hot sliding-window reductions over the 128 edges
         into PSUM rows (full-rate at 512-wide blocks): accT = sum(T),
         accP = sum(phi).
  finale (DVE custom): w = min(|accT*C0 + C1 - accP| / 2pi, 1) with
         per-partition C0/C1 selecting the row-pair's sign encoding
         (Pool ge rows: C0=pi, C1=-64pi; ACT Sign rows: C0=pi/2, C1=0).
Host side pre-scales Sd/Rd by c1^2, and nudges Pc by 1 ulp wherever
f32(Qc+Pc) would be exactly/nearly zero (kills the NaN path of the
BITWISE_NOT seed; ~0-2 pixels per run, winding effect < 1e-7).

Sharding: 8 cores; core c handles batch c//4, image rows [(c%4)*64, +64).
"""
import sys

sys.path.insert(0, "/opt/trn_rl_repo")

import numpy as np

SIZE = 256
B = 2
NPTS = 128
N_CORES = 8
ROWS_CORE = SIZE * SIZE * B // (N_CORES * SIZE)  # 64 image rows per core
RPB = 8                        # rows per superblock
SBLK = RPB * SIZE              # 2048 px
NSBLK = ROWS_CORE // RPB       # 8
BLK = 512                      # pixels per reduction block (one PSUM bank)
NBLK = ROWS_CORE * SIZE // BLK  # 32
ROWS_PER_BLK = BLK // SIZE     # 2 image rows per PSUM row

# tuned on a +-[1,2) mantissa grid: max rel err 0.224% for the 1-NR
# BITWISE_NOT reciprocal z*(1-x*z) with output scale C1SQ folded into Sd/Rd
C0P = -0.11853305
C1SQ = 4.00896949

# row-pair sign-pass flavor: "act" -> ACT Tanh (exact soft sign, pi/2
# encoding), "dve" -> DVE is_gt, else Pool is_gt ({0,1}, pi encoding).
# Quotas chosen to equalize engine busy time (ACT ~7 pairs, DVE 2 pairs).
def _pair_kind(p):
    if p % 4 == 1 and p // 4 != 3:
        return "act"          # 7 pairs
    if p in (7, 23):
        return "dve"          # 2 pairs
    return "pool"             # 23 pairs
PAIR_KIND = [_pair_kind(p) for p in range(NBLK)]
K_TANH = 100000.0

_compiled = {}
_ops = {}


def _register_ops():
    if _ops:
        return _ops
    from concourse import dve_ops
    from concourse.dve_spec import (
        Spec, Src0, Src1, C0, C1, C2, One, minn, lower, AluOp, Bin)
    from concourse.dve_uop import DveOpSpec

    def reg(name, spec):
        if name in dve_ops._SUB_OPCODE_FOR_NAME:
            return next(op for op in dve_ops.OPS if op.name == name)
        row = dve_ops._CUSTOM_DVE_ROW_BASE + len(dve_ops.OPS)
        sha = {ver: DveOpSpec(name=name, opcode=row,
                              uops=lower(spec, ver=ver), rd1_en=True).sha(ver)
               for ver in ("v3", "v4")}
        op = dve_ops.DveOp(name, spec, subdim=False, uops_sha=sha)
        dve_ops.OPS.append(op)
        dve_ops.CUSTOM_DVE_SPECS[name] = spec
        dve_ops._SUB_OPCODE_FOR_NAME[name] = row
        return op

    f32 = np.float32

    # FUSEDQ: q = (Src1 + C1) * (z*(1-x*z)), x = Src0 + C0, z = NOT(x)*C2
    _x = Src0 + C0
    _z = Bin(AluOp.BITWISE_NOT, _x, _x) * C2
    _m = _z * (One - _x * _z)
    _body_q = (Src1 + C1) * _m

    def _ref_q(in0, in1, s0, s1, imm2):
        x = (in0 + s0).astype(f32)
        z = ((~x.view(np.int32)).view(f32) * f32(imm2)).astype(f32)
        m = (z * (f32(1.0) - (x * z).astype(f32)).astype(f32)).astype(f32)
        return ((in1 + s1).astype(f32) * m).astype(f32)

    _ops["q"] = reg("FUSEDQ_V8", Spec(body=_body_q, reference=_ref_q))

    # FIN2: w = min(|Src0*C0 + C1 - Src1| * C2, 1)
    _b = Src0 * C0 + C1
    _body_f = minn(Bin(AluOp.ABSOLUTE_DIFF, _b, Src1) * C2, One)

    def _ref_f(in0, in1, s0, s1, imm2):
        return np.minimum(
            np.abs((in0 * s0 + s1) - in1) * f32(imm2), f32(1.0)).astype(f32)

    _ops["fin"] = reg("FIN2_V8", Spec(body=_body_f, reference=_ref_f))
    return _ops


def _build():
    import concourse.bacc as bacc
    import concourse.tile as tile
    import concourse.mybir as mybir

    AF = mybir.ActivationFunctionType
    ALU = mybir.AluOpType
    f32 = mybir.dt.float32
    f32r = mybir.dt.float32r
    ops = _register_ops()

    nc = bacc.Bacc("TRN2", target_bir_lowering=False, debug=False,
                   num_devices=N_CORES)

    qc_d = nc.dram_tensor("qc", [NPTS, SIZE], f32, kind="ExternalInput").ap()
    sd2_d = nc.dram_tensor("sd2", [NPTS, SIZE], f32, kind="ExternalInput").ap()
    pc_d = nc.dram_tensor("pc", [NPTS, ROWS_CORE], f32, kind="ExternalInput").ap()
    npc_d = nc.dram_tensor("npc", [NPTS, ROWS_CORE], f32, kind="ExternalInput").ap()
    pck_d = nc.dram_tensor("pck", [NPTS, ROWS_CORE], f32, kind="ExternalInput").ap()
    rd2_d = nc.dram_tensor("rd2", [NPTS, ROWS_CORE], f32, kind="ExternalInput").ap()
    redg_d = nc.dram_tensor("redg", [NPTS, 63], f32r, kind="ExternalInput").ap()
    fc0_d = nc.dram_tensor("fc0", [NBLK, 1], f32, kind="ExternalInput").ap()
    fc1_d = nc.dram_tensor("fc1", [NBLK, 1], f32, kind="ExternalInput").ap()
    out_d = nc.dram_tensor("out", [NBLK, BLK], f32, kind="ExternalOutput").ap()

    with tile.TileContext(nc) as tc:
        with tc.tile_pool(name="cst", bufs=1) as cst, \
             tc.tile_pool(name="work", bufs=3) as work, \
             tc.tile_pool(name="pacc", bufs=1, space="PSUM") as pacc:
            qc_t = cst.tile([NPTS, SIZE], f32, name="qc_t")
            sd2_t = cst.tile([NPTS, SIZE], f32, name="sd2_t")
            pc_t = cst.tile([NPTS, ROWS_CORE], f32, name="pc_t")
            npc_t = cst.tile([NPTS, ROWS_CORE], f32, name="npc_t")
            pck_t = cst.tile([NPTS, ROWS_CORE], f32, name="pck_t")
            rd2_t = cst.tile([NPTS, ROWS_CORE], f32, name="rd2_t")
            redg_t = cst.tile([NPTS, 63], f32r, name="redg_t")
            fc0_t = cst.tile([NBLK, 1], f32, name="fc0_t")
            fc1_t = cst.tile([NBLK, 1], f32, name="fc1_t")
            # ordered by first use: fusedq needs qc/sd2/pc/rd2, then the
            # sign-pass needs npc/pck, then weights and finale consts
            nc.sync.dma_start(pc_t[:], pc_d[:])
            nc.sync.dma_start(rd2_t[:], rd2_d[:])
            nc.sync.dma_start(qc_t[:], qc_d[:])
            nc.sync.dma_start(sd2_t[:], sd2_d[:])
            nc.sync.dma_start(npc_t[:], npc_d[:])
            nc.sync.dma_start(pck_t[:], pck_d[:])
            nc.sync.dma_start(redg_t[:], redg_d[:])
            nc.sync.dma_start(fc0_t[:], fc0_d[:])
            nc.sync.dma_start(fc1_t[:], fc1_d[:])

            accT = pacc.tile([NBLK, BLK], f32, name="accT")
            accP = pacc.tile([NBLK, BLK], f32, name="accP")

            def emit_front(u):
                """DVE fused-q + the whole sign pass for superblock u."""
                q = work.tile([NPTS, SBLK], f32, tag="q", name=f"q{u}")
                for h in range(RPB):
                    i = u * RPB + h
                    hs = slice(h * SIZE, (h + 1) * SIZE)
                    nc.vector._custom_dve(
                        ops["q"], out=q[:, hs], in0=qc_t[:], in1=sd2_t[:],
                        s0=pc_t[:, i:i + 1], s1=rd2_t[:, i:i + 1], imm2=C0P)
                g = work.tile([NPTS, SBLK], f32r, tag="g", name=f"g{u}")
                for h in range(RPB):
                    i = u * RPB + h
                    hs = slice(h * SIZE, (h + 1) * SIZE)
                    kind = PAIR_KIND[i // ROWS_PER_BLK]
                    if kind == "act":
                        nc.scalar.activation(g[:, hs], qc_t[:], AF.Tanh,
                                             scale=K_TANH,
                                             bias=pck_t[:, i:i + 1])
                    elif kind == "dve":
                        nc.vector.tensor_scalar(g[:, hs], qc_t[:],
                                                npc_t[:, i:i + 1], None,
                                                ALU.is_gt)
                    else:
                        nc.gpsimd.tensor_scalar(g[:, hs], qc_t[:],
                                                npc_t[:, i:i + 1], None,
                                                ALU.is_gt)
                # sum(g) matmuls can start as soon as g is written (phi is
                # still in flight) - emit them ahead of the arctan
                for h2 in range(SBLK // BLK):
                    j = (SBLK // BLK) * u + h2
                    hs = slice(h2 * BLK, (h2 + 1) * BLK)
                    nc.tensor.matmul(accT[:], redg_t[:, 31 - j:63 - j],
                                     g[:, hs],
                                     start=(j == 0), stop=(j == NBLK - 1))
                return q

            def emit_back(u, q):
                """Arctan + phi reduction for superblock u."""
                phi = work.tile([NPTS, SBLK], f32r, tag="phi", name=f"phi{u}")
                nc.scalar.activation(phi[:], q[:], AF.Arctan)
                for h2 in range(SBLK // BLK):
                    j = (SBLK // BLK) * u + h2
                    hs = slice(h2 * BLK, (h2 + 1) * BLK)
                    nc.tensor.matmul(accP[:], redg_t[:, 31 - j:63 - j],
                                     phi[:, hs],
                                     start=(j == 0), stop=(j == NBLK - 1))

            prev = None
            for u in range(NSBLK):
                q = emit_front(u)
                if prev is not None:
                    emit_back(*prev)
                prev = (u, q)
            emit_back(*prev)

            pcopy = work.tile([NBLK, BLK], f32, tag="pcopy", name="pcopy")
            nc.scalar.copy(pcopy[:], accP[:])
            w = work.tile([NBLK, BLK], f32, tag="w", name="w")
            nc.vector._custom_dve(
                ops["fin"], out=w[:], in0=accT[:], in1=pcopy[:],
                s0=fc0_t[:], s1=fc1_t[:],
                imm2=float(np.float32(1.0 / (2.0 * np.pi))))
            nc.sync.dma_start(out_d[:], w[:])

    nc.compile()
    return nc


def _host_inputs(contour: np.ndarray):
    """Per-core in_maps from the full (B, NPTS, 2) contour."""
    f32 = np.float32
    mx = (np.arange(SIZE) / SIZE).astype(np.float64)
    my = mx

    prof = []
    for b in range(B):
        cx = contour[b, :, 0].astype(np.float64)
        cy = contour[b, :, 1].astype(np.float64)
        cxn = np.roll(cx, -1)
        cyn = np.roll(cy, -1)
        A = cy * cxn - cx * cyn
        Bc = cyn - cy
        Cc = cx - cxn
        Dd = cx * cxn + cy * cyn
        Ed = -(cx + cxn)
        Fd = -(cy + cyn)
        Pc = (A[:, None] + Bc[:, None] * mx[None, :]).astype(f32)
        Qc = (Cc[:, None] * my[None, :]).astype(f32)
        Rd2 = ((Dd[:, None] + Ed[:, None] * mx[None, :] + mx[None, :] ** 2)
               * C1SQ).astype(f32)
        Sd2 = ((Fd[:, None] * my[None, :] + my[None, :] ** 2) * C1SQ).astype(f32)
        # zero-exterminator: f32(Qc + Pc_i) == +-0 would NaN the NOT seed.
        for _ in range(4):
            cross = Qc[:, None, :] + Pc[:, :, None]  # f32 [N, i, j]
            n_, i_ = np.nonzero((np.abs(cross) < 1e-30).any(axis=2))
            if len(n_) == 0:
                break
            Pc[n_, i_] = np.nextafter(Pc[n_, i_], f32(np.inf), dtype=f32)
        prof.append((Pc, Qc, Rd2, Sd2))

    redg = np.zeros((NPTS, 63), dtype=f32)
    redg[:, 31] = 1.0
    is_act = np.array([k == "act" for k in PAIR_KIND])
    fc0 = np.where(is_act, f32(np.pi / 2), f32(np.pi)).astype(f32)[:, None]
    fc1 = np.where(is_act, f32(0.0), f32(-64.0 * np.pi)).astype(f32)[:, None]

    in_maps = []
    for c in range(N_CORES):
        b = c // (N_CORES // B)
        r0 = (c % (N_CORES // B)) * ROWS_CORE
        Pc, Qc, Rd2, Sd2 = prof[b]
        pc = np.ascontiguousarray(Pc[:, r0:r0 + ROWS_CORE])
        in_maps.append({
            "qc": Qc,
            "sd2": Sd2,
            "pc": pc,
            "npc": -pc,
            "pck": (pc.astype(np.float64) * K_TANH).astype(f32),
            "rd2": np.ascontiguousarray(Rd2[:, r0:r0 + ROWS_CORE]),
            "redg": redg,
            "fc0": fc0,
            "fc1": fc1,
        })
    return in_maps


def kernel(contour: np.ndarray) -> np.ndarray:
    from concourse import bass_utils

    contour = np.asarray(contour, dtype=np.float32)
    if "nc" not in _compiled:
        _compiled["nc"] = _build()
    in_maps = _host_inputs(contour)
    res = bass_utils.run_bass_kernel_spmd(
        _compiled["nc"], in_maps, core_ids=list(range(N_CORES))).results

    mask = np.zeros((1, B, SIZE, SIZE), dtype=np.float32)
    for c in range(N_CORES):
        b = c // (N_CORES // B)
        r0 = (c % (N_CORES // B)) * ROWS_CORE
        mask[0, b, r0:r0 + ROWS_CORE, :] = (
            res[c]["out"].reshape(ROWS_CORE, SIZE))
    return mask


# revision 8
# speedup vs baseline: 1.8355x; 1.0514x over previous
"""Trainium2 Bass kernel for nn_Contour_to_mask (winding-number soft
rasterization of a 128-point contour into a (1, 2, 256, 256) f32 mask).

v8 "three-pass" design. Math: for pixel (i,j) and edge n,
  cross = Pc[n,i] + Qc[n,j],  dot = Rd[n,i] + Sd[n,j]   (separable profiles)
  contribution = s*(pi/2 - arctan(dot/|cross|)) with s = tanh(1e5*cross)
Approximated (rel-err ~1.4e-2 < 2e-2 gate, verified vs reference) as
  contribution = (pi/2)*sgn(cross) - arctan(dot/cross)
so the per-element work collapses to THREE elementwise passes:
  DVE :  one fused 7-stage custom op per image row:
           x = Qc + Pc_i                 (cross, built inline)
           z = BITWISE_NOT(x) * c0'      (fast-reciprocal seed)
           m = z*(1 - x*z)               (1 Newton step; ~0.22% rel err
                                          with c1^2 pre-folded into Sd/Rd)
           q = (Sd'' + Rd''_i) * m       (signed dot/cross, unclipped)
  ACT :  phi = Arctan(q) in f32 (one big call per superblock), plus a
         share of the sign pass as Sign(Qc*1 + Pc_i) (inline bias).
  Pool:  rest of the sign pass as tensor_scalar is_gt: g = (Qc > -Pc_i).
  PE  :  two fp32r one-hot sliding-window reductions over the 128 edges
         into PSUM rows (full-rate at 512-wide blocks): accT = sum(T),
         accP = sum(phi).
  finale (DVE custom): w = min(|accT*C0 + C1 - accP| / 2pi, 1) with
         per-partition C0/C1 selecting the row-pair's sign encoding
         (Pool ge rows: C0=pi, C1=-64pi; ACT Sign rows: C0=pi/2, C1=0).
Host side pre-scales Sd/Rd by c1^2, and nudges Pc by 1 ulp wherever
f32(Qc+Pc) would be exactly/nearly zero (kills the NaN path of the
BITWISE_NOT seed; ~0-2 pixels per run, winding effect < 1e-7).

Sharding: 8 cores; core c handles batch c//4, image rows [(c%4)*64, +64).
"""
import sys

sys.path.insert(0, "/opt/trn_rl_repo")

import numpy as np

SIZE = 256
B = 2
NPTS = 128
N_CORES = 8
ROWS_CORE = SIZE * SIZE * B // (N_CORES * SIZE)  # 64 image rows per core
RPB = 8                        # rows per superblock
SBLK = RPB * SIZE              # 2048 px
NSBLK = ROWS_CORE // RPB       # 8
BLK = 512                      # pixels per reduction block (one PSUM bank)
NBLK = ROWS_CORE * SIZE // BLK  # 32
ROWS_PER_BLK = BLK // SIZE     # 2 image rows per PSUM row

# tuned on a +-[1,2) mantissa grid: max rel err 0.224% for the 1-NR
# BITWISE_NOT reciprocal z*(1-x*z) with output scale C1SQ folded into Sd/Rd
C0P = -0.11853305
C1SQ = 4.00896949

# row-pair sign-pass flavor: "act" -> ACT Tanh (exact soft sign, pi/2
# encoding), "dve" -> DVE is_gt, else Pool is_gt ({0,1}, pi encoding).
# Quotas chosen to equalize engine busy time (ACT ~7 pairs, DVE 2 pairs).
def _pair_kind(p):
    if p % 4 == 1 and p // 4 != 3:
        return "act"          # 7 pairs
    if p in (7, 23):
        return "dve"          # 2 pairs
    return "pool"             # 23 pairs
PAIR_KIND = [_pair_kind(p) for p in range(NBLK)]
K_TANH = 100000.0

_compiled = {}
_ops = {}


def _register_ops():
    if _ops:
        return _ops
    from concourse import dve_ops
    from concourse.dve_spec import (
        Spec, Src0, Src1, C0, C1, C2, One, minn, lower, AluOp, Bin)
    from concourse.dve_uop import DveOpSpec

    def reg(name, spec):
        if name in dve_ops._SUB_OPCODE_FOR_NAME:
            return next(op for op in dve_ops.OPS if op.name == name)
        row = dve_ops._CUSTOM_DVE_ROW_BASE + len(dve_ops.OPS)
        sha = {ver: DveOpSpec(name=name, opcode=row,
                              uops=lower(spec, ver=ver), rd1_en=True).sha(ver)
               for ver in ("v3", "v4")}
        op = dve_ops.DveOp(name, spec, subdim=False, uops_sha=sha)
        dve_ops.OPS.append(op)
        dve_ops.CUSTOM_DVE_SPECS[name] = spec
        dve_ops._SUB_OPCODE_FOR_NAME[name] = row
        return op

    f32 = np.float32

    # FUSEDQ: q = (Src1 + C1) * (z*(1-x*z)), x = Src0 + C0, z = NOT(x)*C2
    _x = Src0 + C0
    _z = Bin(AluOp.BITWISE_NOT, _x, _x) * C2
    _m = _z * (One - _x * _z)
    _body_q = (Src1 + C1) * _m

    def _ref_q(in0, in1, s0, s1, imm2):
        x = (in0 + s0).astype(f32)
        z = ((~x.view(np.int32)).view(f32) * f32(imm2)).astype(f32)
        m = (z * (f32(1.0) - (x * z).astype(f32)).astype(f32)).astype(f32)
        return ((in1 + s1).astype(f32) * m).astype(f32)

    _ops["q"] = reg("FUSEDQ_V8", Spec(body=_body_q, reference=_ref_q))

    # FIN2: w = min(|Src0*C0 + C1 - Src1| * C2, 1)
    _b = Src0 * C0 + C1
    _body_f = minn(Bin(AluOp.ABSOLUTE_DIFF, _b, Src1) * C2, One)

    def _ref_f(in0, in1, s0, s1, imm2):
        return np.minimum(
            np.abs((in0 * s0 + s1) - in1) * f32(imm2), f32(1.0)).astype(f32)

    _ops["fin"] = reg("FIN2_V8", Spec(body=_body_f, reference=_ref_f))
    return _ops


def _build():
    import concourse.bacc as bacc
    import concourse.tile as tile
    import concourse.mybir as mybir

    AF = mybir.ActivationFunctionType
    ALU = mybir.AluOpType
    f32 = mybir.dt.float32
    f32r = mybir.dt.float32r
    ops = _register_ops()

    nc = bacc.Bacc("TRN2", target_bir_lowering=False, debug=False,
                   num_devices=N_CORES)

    # prof = [pc | npc | pck | rd2], cols = [qc | sd2], fc = [fc0 | fc1]
    prof_d = nc.dram_tensor("prof", [NPTS, 4 * ROWS_CORE], f32,
                            kind="ExternalInput").ap()
    cols_d = nc.dram_tensor("cols", [NPTS, 2 * SIZE], f32,
                            kind="ExternalInput").ap()
    redg_d = nc.dram_tensor("redg", [NPTS, 63], f32r, kind="ExternalInput").ap()
    fc_d = nc.dram_tensor("fc", [NBLK, 2], f32, kind="ExternalInput").ap()
    out_d = nc.dram_tensor("out", [NBLK, BLK], f32, kind="ExternalOutput").ap()

    with tile.TileContext(nc) as tc:
        with tc.tile_pool(name="cst", bufs=1) as cst, \
             tc.tile_pool(name="work", bufs=3) as work, \
             tc.tile_pool(name="pacc", bufs=1, space="PSUM") as pacc:
            prof_t = cst.tile([NPTS, 4 * ROWS_CORE], f32, name="prof_t")
            cols_t = cst.tile([NPTS, 2 * SIZE], f32, name="cols_t")
            redg_t = cst.tile([NPTS, 63], f32r, name="redg_t")
            fc_t = cst.tile([NBLK, 2], f32, name="fc_t")
            nc.sync.dma_start(prof_t[:], prof_d[:])
            nc.sync.dma_start(cols_t[:], cols_d[:])
            nc.sync.dma_start(redg_t[:], redg_d[:])
            nc.sync.dma_start(fc_t[:], fc_d[:])
            qc_t = cols_t[:, 0:SIZE]
            sd2_t = cols_t[:, SIZE:2 * SIZE]
            pc_t = prof_t[:, 0 * ROWS_CORE:1 * ROWS_CORE]
            npc_t = prof_t[:, 1 * ROWS_CORE:2 * ROWS_CORE]
            pck_t = prof_t[:, 2 * ROWS_CORE:3 * ROWS_CORE]
            rd2_t = prof_t[:, 3 * ROWS_CORE:4 * ROWS_CORE]

            accT = pacc.tile([NBLK, BLK], f32, name="accT")
            accP = pacc.tile([NBLK, BLK], f32, name="accP")

            def emit_front(u):
                """DVE fused-q + the whole sign pass for superblock u."""
                q = work.tile([NPTS, SBLK], f32, tag="q", name=f"q{u}")
                for h in range(RPB):
                    i = u * RPB + h
                    hs = slice(h * SIZE, (h + 1) * SIZE)
                    nc.vector._custom_dve(
                        ops["q"], out=q[:, hs], in0=qc_t, in1=sd2_t,
                        s0=pc_t[:, i:i + 1], s1=rd2_t[:, i:i + 1], imm2=C0P)
                g = work.tile([NPTS, SBLK], f32r, tag="g", name=f"g{u}")
                for h in range(RPB):
                    i = u * RPB + h
                    hs = slice(h * SIZE, (h + 1) * SIZE)
                    kind = PAIR_KIND[i // ROWS_PER_BLK]
                    if kind == "act":
                        nc.scalar.activation(g[:, hs], qc_t, AF.Tanh,
                                             scale=K_TANH,
                                             bias=pck_t[:, i:i + 1])
                    elif kind == "dve":
                        nc.vector.tensor_scalar(g[:, hs], qc_t,
                                                npc_t[:, i:i + 1], None,
                                                ALU.is_gt)
                    else:
                        nc.gpsimd.tensor_scalar(g[:, hs], qc_t,
                                                npc_t[:, i:i + 1], None,
                                                ALU.is_gt)
                # sum(g) matmuls can start as soon as g is written (phi is
                # still in flight) - emit them ahead of the arctan
                for h2 in range(SBLK // BLK):
                    j = (SBLK // BLK) * u + h2
                    hs = slice(h2 * BLK, (h2 + 1) * BLK)
                    nc.tensor.matmul(accT[:], redg_t[:, 31 - j:63 - j],
                                     g[:, hs],
                                     start=(j == 0), stop=(j == NBLK - 1))
                return q

            def emit_back(u, q):
                """Arctan + phi reduction for superblock u."""
                phi = work.tile([NPTS, SBLK], f32r, tag="phi", name=f"phi{u}")
                nc.scalar.activation(phi[:], q[:], AF.Arctan)
                for h2 in range(SBLK // BLK):
                    j = (SBLK // BLK) * u + h2
                    hs = slice(h2 * BLK, (h2 + 1) * BLK)
                    nc.tensor.matmul(accP[:], redg_t[:, 31 - j:63 - j],
                                     phi[:, hs],
                                     start=(j == 0), stop=(j == NBLK - 1))

            prev = None
            for u in range(NSBLK):
                q = emit_front(u)
                if prev is not None:
                    emit_back(*prev)
                prev = (u, q)
            emit_back(*prev)

            pcopy = work.tile([NBLK, BLK], f32, tag="pcopy", name="pcopy")
            nc.scalar.copy(pcopy[:], accP[:])
            w = work.tile([NBLK, BLK], f32, tag="w", name="w")
            nc.vector._custom_dve(
                ops["fin"], out=w[:], in0=accT[:], in1=pcopy[:],
                s0=fc_t[:, 0:1], s1=fc_t[:, 1:2],
                imm2=float(np.float32(1.0 / (2.0 * np.pi))))
            nc.sync.dma_start(out_d[:], w[:])

    nc.compile()
    return nc


def _host_inputs(contour: np.ndarray):
    """Per-core in_maps from the full (B, NPTS, 2) contour."""
    f32 = np.float32
    mx = (np.arange(SIZE) / SIZE).astype(np.float64)
    my = mx

    prof = []
    for b in range(B):
        cx = contour[b, :, 0].astype(np.float64)
        cy = contour[b, :, 1].astype(np.float64)
        cxn = np.roll(cx, -1)
        cyn = np.roll(cy, -1)
        A = cy * cxn - cx * cyn
        Bc = cyn - cy
        Cc = cx - cxn
        Dd = cx * cxn + cy * cyn
        Ed = -(cx + cxn)
        Fd = -(cy + cyn)
        Pc = (A[:, None] + Bc[:, None] * mx[None, :]).astype(f32)
        Qc = (Cc[:, None] * my[None, :]).astype(f32)
        Rd2 = ((Dd[:, None] + Ed[:, None] * mx[None, :] + mx[None, :] ** 2)
               * C1SQ).astype(f32)
        Sd2 = ((Fd[:, None] * my[None, :] + my[None, :] ** 2) * C1SQ).astype(f32)
        # zero-exterminator: f32(Qc + Pc_i) == +-0 would NaN the NOT seed.
        for _ in range(4):
            cross = Qc[:, None, :] + Pc[:, :, None]  # f32 [N, i, j]
            n_, i_ = np.nonzero((np.abs(cross) < 1e-30).any(axis=2))
            if len(n_) == 0:
                break
            Pc[n_, i_] = np.nextafter(Pc[n_, i_], f32(np.inf), dtype=f32)
        prof.append((Pc, Qc, Rd2, Sd2))

    redg = np.zeros((NPTS, 63), dtype=f32)
    redg[:, 31] = 1.0
    is_act = np.array([k == "act" for k in PAIR_KIND])
    fc0 = np.where(is_act, f32(np.pi / 2), f32(np.pi)).astype(f32)[:, None]
    fc1 = np.where(is_act, f32(0.0), f32(-64.0 * np.pi)).astype(f32)[:, None]

    in_maps = []
    for c in range(N_CORES):
        b = c // (N_CORES // B)
        r0 = (c % (N_CORES // B)) * ROWS_CORE
        Pc, Qc, Rd2, Sd2 = prof[b]
        pc = Pc[:, r0:r0 + ROWS_CORE]
        prof_blob = np.concatenate(
            [pc, -pc, (pc.astype(np.float64) * K_TANH).astype(f32),
             Rd2[:, r0:r0 + ROWS_CORE]], axis=1)
        in_maps.append({
            "prof": np.ascontiguousarray(prof_blob),
            "cols": np.ascontiguousarray(np.concatenate([Qc, Sd2], axis=1)),
            "redg": redg,
            "fc": np.ascontiguousarray(np.concatenate([fc0, fc1], axis=1)),
        })
    return in_maps


def kernel(contour: np.ndarray) -> np.ndarray:
    from concourse import bass_utils

    contour = np.asarray(contour, dtype=np.float32)
    if "nc" not in _compiled:
        _compiled["nc"] = _build()
    in_maps = _host_inputs(contour)
    res = bass_utils.run_bass_kernel_spmd(
        _compiled["nc"], in_maps, core_ids=list(range(N_CORES))).results

    mask = np.zeros((1, B, SIZE, SIZE), dtype=np.float32)
    for c in range(N_CORES):
        b = c // (N_CORES // B)
        r0 = (c % (N_CORES // B)) * ROWS_CORE
        mask[0, b, r0:r0 + ROWS_CORE, :] = (
            res[c]["out"].reshape(ROWS_CORE, SIZE))
    return mask


# revision 10
# speedup vs baseline: 1.8565x; 1.0114x over previous
"""Trainium2 Bass kernel for nn_Contour_to_mask (winding-number soft
rasterization of a 128-point contour into a (1, 2, 256, 256) f32 mask).

v8 "three-pass" design. Math: for pixel (i,j) and edge n,
  cross = Pc[n,i] + Qc[n,j],  dot = Rd[n,i] + Sd[n,j]   (separable profiles)
  contribution = s*(pi/2 - arctan(dot/|cross|)) with s = tanh(1e5*cross)
Approximated (rel-err ~1.4e-2 < 2e-2 gate, verified vs reference) as
  contribution = (pi/2)*sgn(cross) - arctan(dot/cross)
so the per-element work collapses to THREE elementwise passes:
  DVE :  one fused 7-stage custom op per image row:
           x = Qc + Pc_i                 (cross, built inline)
           z = BITWISE_NOT(x) * c0'      (fast-reciprocal seed)
           m = z*(1 - x*z)               (1 Newton step; ~0.22% rel err
                                          with c1^2 pre-folded into Sd/Rd)
           q = (Sd'' + Rd''_i) * m       (signed dot/cross, unclipped)
  ACT :  phi = Arctan(q) in f32 (one big call per superblock), plus a
         share of the sign pass as Sign(Qc*1 + Pc_i) (inline bias).
  Pool:  rest of the sign pass as tensor_scalar is_gt: g = (Qc > -Pc_i).
  PE  :  two fp32r one-hot sliding-window reductions over the 128 edges
         into PSUM rows (full-rate at 512-wide blocks): accT = sum(T),
         accP = sum(phi).
  finale (DVE custom): w = min(|accT*C0 + C1 - accP| / 2pi, 1) with
         per-partition C0/C1 selecting the row-pair's sign encoding
         (Pool ge rows: C0=pi, C1=-64pi; ACT Sign rows: C0=pi/2, C1=0).
Host side pre-scales Sd/Rd by c1^2, and nudges Pc by 1 ulp wherever
f32(Qc+Pc) would be exactly/nearly zero (kills the NaN path of the
BITWISE_NOT seed; ~0-2 pixels per run, winding effect < 1e-7).

Sharding: 8 cores; core c handles batch c//4, image rows [(c%4)*64, +64).
"""
import sys

sys.path.insert(0, "/opt/trn_rl_repo")

import numpy as np

SIZE = 256
B = 2
NPTS = 128
N_CORES = 8
ROWS_CORE = SIZE * SIZE * B // (N_CORES * SIZE)  # 64 image rows per core
RPB = 8                        # rows per superblock
SBLK = RPB * SIZE              # 2048 px
NSBLK = ROWS_CORE // RPB       # 8
BLK = 512                      # pixels per reduction block (one PSUM bank)
NBLK = ROWS_CORE * SIZE // BLK  # 32
ROWS_PER_BLK = BLK // SIZE     # 2 image rows per PSUM row

# tuned on a +-[1,2) mantissa grid: max rel err 0.224% for the 1-NR
# BITWISE_NOT reciprocal z*(1-x*z) with output scale C1SQ folded into Sd/Rd
C0P = -0.11853305
C1SQ = 4.00896949

# row-pair sign-pass flavor: "act" -> ACT Tanh (exact soft sign, pi/2
# encoding), "dve" -> DVE is_gt, else Pool is_gt ({0,1}, pi encoding).
# Quotas chosen to equalize engine busy time (ACT ~7 pairs, DVE 2 pairs).
def _pair_kind(p):
    if p % 4 == 1 and p // 4 != 3:
        return "act"          # 7 pairs
    if p in (7, 23):
        return "dve"          # 2 pairs
    return "pool"             # 23 pairs
PAIR_KIND = [_pair_kind(p) for p in range(NBLK)]
K_TANH = 100000.0

_compiled = {}
_ops = {}


def _register_ops():
    if _ops:
        return _ops
    from concourse import dve_ops
    from concourse.dve_spec import (
        Spec, Src0, Src1, C0, C1, C2, One, minn, lower, AluOp, Bin)
    from concourse.dve_uop import DveOpSpec

    def reg(name, spec):
        if name in dve_ops._SUB_OPCODE_FOR_NAME:
            return next(op for op in dve_ops.OPS if op.name == name)
        row = dve_ops._CUSTOM_DVE_ROW_BASE + len(dve_ops.OPS)
        sha = {ver: DveOpSpec(name=name, opcode=row,
                              uops=lower(spec, ver=ver), rd1_en=True).sha(ver)
               for ver in ("v3", "v4")}
        op = dve_ops.DveOp(name, spec, subdim=False, uops_sha=sha)
        dve_ops.OPS.append(op)
        dve_ops.CUSTOM_DVE_SPECS[name] = spec
        dve_ops._SUB_OPCODE_FOR_NAME[name] = row
        return op

    f32 = np.float32

    # FUSEDQ: q = (Src1 + C1) * (z*(1-x*z)), x = Src0 + C0, z = NOT(x)*C2
    _x = Src0 + C0
    _z = Bin(AluOp.BITWISE_NOT, _x, _x) * C2
    _m = _z * (One - _x * _z)
    _body_q = (Src1 + C1) * _m

    def _ref_q(in0, in1, s0, s1, imm2):
        x = (in0 + s0).astype(f32)
        z = ((~x.view(np.int32)).view(f32) * f32(imm2)).astype(f32)
        m = (z * (f32(1.0) - (x * z).astype(f32)).astype(f32)).astype(f32)
        return ((in1 + s1).astype(f32) * m).astype(f32)

    _ops["q"] = reg("FUSEDQ_V8", Spec(body=_body_q, reference=_ref_q))

    # FIN2: w = min(|Src0*C0 + C1 - Src1| * C2, 1)
    _b = Src0 * C0 + C1
    _body_f = minn(Bin(AluOp.ABSOLUTE_DIFF, _b, Src1) * C2, One)

    def _ref_f(in0, in1, s0, s1, imm2):
        return np.minimum(
            np.abs((in0 * s0 + s1) - in1) * f32(imm2), f32(1.0)).astype(f32)

    _ops["fin"] = reg("FIN2_V8", Spec(body=_body_f, reference=_ref_f))
    return _ops


def _build():
    import concourse.bacc as bacc
    import concourse.tile as tile
    import concourse.mybir as mybir

    AF = mybir.ActivationFunctionType
    ALU = mybir.AluOpType
    f32 = mybir.dt.float32
    f32r = mybir.dt.float32r
    ops = _register_ops()

    nc = bacc.Bacc("TRN2", target_bir_lowering=False, debug=False,
                   num_devices=N_CORES)

    # blob = [pc | npc | pck | rd2 | fc0 | fc1 | qc | sd2] along free dim
    BW = 4 * ROWS_CORE + 2 + 2 * SIZE
    blob_d = nc.dram_tensor("blob", [NPTS, BW], f32, kind="ExternalInput").ap()
    redg_d = nc.dram_tensor("redg", [NPTS, 2 * NBLK - 1], f32r,
                            kind="ExternalInput").ap()
    out_d = nc.dram_tensor("out", [NBLK, BLK], f32, kind="ExternalOutput").ap()

    with tile.TileContext(nc) as tc:
        with tc.tile_pool(name="cst", bufs=1) as cst, \
             tc.tile_pool(name="work", bufs=3) as work, \
             tc.tile_pool(name="pacc", bufs=1, space="PSUM") as pacc:
            blob_t = cst.tile([NPTS, BW], f32, name="blob_t")
            redg_t = cst.tile([NPTS, 2 * NBLK - 1], f32r, name="redg_t")
            nc.sync.dma_start(blob_t[:], blob_d[:])
            nc.sync.dma_start(redg_t[:], redg_d[:])
            pc_t = blob_t[:, 0 * ROWS_CORE:1 * ROWS_CORE]
            npc_t = blob_t[:, 1 * ROWS_CORE:2 * ROWS_CORE]
            pck_t = blob_t[:, 2 * ROWS_CORE:3 * ROWS_CORE]
            rd2_t = blob_t[:, 3 * ROWS_CORE:4 * ROWS_CORE]
            fc_t = blob_t[:, 4 * ROWS_CORE:4 * ROWS_CORE + 2]
            qc_t = blob_t[:, 4 * ROWS_CORE + 2:4 * ROWS_CORE + 2 + SIZE]
            sd2_t = blob_t[:, 4 * ROWS_CORE + 2 + SIZE:BW]

            accT = pacc.tile([NBLK, BLK], f32, name="accT")
            accP = pacc.tile([NBLK, BLK], f32, name="accP")

            def emit_front(u):
                """DVE fused-q + the whole sign pass for superblock u."""
                q = work.tile([NPTS, SBLK], f32, tag="q", name=f"q{u}")
                for h in range(RPB):
                    i = u * RPB + h
                    hs = slice(h * SIZE, (h + 1) * SIZE)
                    nc.vector._custom_dve(
                        ops["q"], out=q[:, hs], in0=qc_t, in1=sd2_t,
                        s0=pc_t[:, i:i + 1], s1=rd2_t[:, i:i + 1], imm2=C0P)
                g = work.tile([NPTS, SBLK], f32r, tag="g", name=f"g{u}")
                for h in range(RPB):
                    i = u * RPB + h
                    hs = slice(h * SIZE, (h + 1) * SIZE)
                    kind = PAIR_KIND[i // ROWS_PER_BLK]
                    if kind == "act":
                        nc.scalar.activation(g[:, hs], qc_t, AF.Tanh,
                                             scale=K_TANH,
                                             bias=pck_t[:, i:i + 1])
                    elif kind == "dve":
                        nc.vector.tensor_scalar(g[:, hs], qc_t,
                                                npc_t[:, i:i + 1], None,
                                                ALU.is_gt)
                    else:
                        nc.gpsimd.tensor_scalar(g[:, hs], qc_t,
                                                npc_t[:, i:i + 1], None,
                                                ALU.is_gt)
                # sum(g) matmuls can start as soon as g is written (phi is
                # still in flight) - emit them ahead of the arctan
                for h2 in range(SBLK // BLK):
                    j = (SBLK // BLK) * u + h2
                    hs = slice(h2 * BLK, (h2 + 1) * BLK)
                    nc.tensor.matmul(accT[:], redg_t[:, NBLK - 1 - j:2 * NBLK - 1 - j],
                                     g[:, hs],
                                     start=(j == 0), stop=(j == NBLK - 1))
                return q

            def emit_back(u, q):
                """Arctan + phi reduction for superblock u."""
                phi = work.tile([NPTS, SBLK], f32r, tag="phi", name=f"phi{u}")
                nc.scalar.activation(phi[:], q[:], AF.Arctan)
                for h2 in range(SBLK // BLK):
                    j = (SBLK // BLK) * u + h2
                    hs = slice(h2 * BLK, (h2 + 1) * BLK)
                    nc.tensor.matmul(accP[:], redg_t[:, NBLK - 1 - j:2 * NBLK - 1 - j],
                                     phi[:, hs],
                                     start=(j == 0), stop=(j == NBLK - 1))

            prev = None
            for u in range(NSBLK):
                q = emit_front(u)
                if prev is not None:
                    emit_back(*prev)
                prev = (u, q)
            emit_back(*prev)

            pcopy = work.tile([NBLK, BLK], f32, tag="pcopy", name="pcopy")
            nc.scalar.copy(pcopy[:], accP[:])
            w = work.tile([NBLK, BLK], f32, tag="w", name="w")
            nc.vector._custom_dve(
                ops["fin"], out=w[:], in0=accT[:], in1=pcopy[:],
                s0=fc_t[0:NBLK, 0:1], s1=fc_t[0:NBLK, 1:2],
                imm2=float(np.float32(1.0 / (2.0 * np.pi))))
            nc.sync.dma_start(out_d[:], w[:])

    nc.compile()
    return nc


def _host_inputs(contour: np.ndarray):
    """Per-core in_maps from the full (B, NPTS, 2) contour."""
    f32 = np.float32
    mx = (np.arange(SIZE) / SIZE).astype(np.float64)
    my = mx

    prof = []
    for b in range(B):
        cx = contour[b, :, 0].astype(np.float64)
        cy = contour[b, :, 1].astype(np.float64)
        cxn = np.roll(cx, -1)
        cyn = np.roll(cy, -1)
        A = cy * cxn - cx * cyn
        Bc = cyn - cy
        Cc = cx - cxn
        Dd = cx * cxn + cy * cyn
        Ed = -(cx + cxn)
        Fd = -(cy + cyn)
        Pc = (A[:, None] + Bc[:, None] * mx[None, :]).astype(f32)
        Qc = (Cc[:, None] * my[None, :]).astype(f32)
        Rd2 = ((Dd[:, None] + Ed[:, None] * mx[None, :] + mx[None, :] ** 2)
               * C1SQ).astype(f32)
        Sd2 = ((Fd[:, None] * my[None, :] + my[None, :] ** 2) * C1SQ).astype(f32)
        # zero-exterminator: f32(Qc + Pc_i) == +-0 would NaN the NOT seed.
        for _ in range(4):
            cross = Qc[:, None, :] + Pc[:, :, None]  # f32 [N, i, j]
            n_, i_ = np.nonzero((np.abs(cross) < 1e-30).any(axis=2))
            if len(n_) == 0:
                break
            Pc[n_, i_] = np.nextafter(Pc[n_, i_], f32(np.inf), dtype=f32)
        prof.append((Pc, Qc, Rd2, Sd2))

    redg = np.zeros((NPTS, 2 * NBLK - 1), dtype=f32)
    redg[:, NBLK - 1] = 1.0
    is_act = np.array([k == "act" for k in PAIR_KIND])
    fc0 = np.where(is_act, f32(np.pi / 2), f32(np.pi)).astype(f32)[:, None]
    fc1 = np.where(is_act, f32(0.0), f32(-64.0 * np.pi)).astype(f32)[:, None]

    in_maps = []
    for c in range(N_CORES):
        b = c // (N_CORES // B)
        r0 = (c % (N_CORES // B)) * ROWS_CORE
        Pc, Qc, Rd2, Sd2 = prof[b]
        pc = Pc[:, r0:r0 + ROWS_CORE]
        fcpad = np.zeros((NPTS, 2), dtype=f32)
        fcpad[:NBLK, 0] = fc0[:, 0]
        fcpad[:NBLK, 1] = fc1[:, 0]
        blob = np.concatenate(
            [pc, -pc, (pc.astype(np.float64) * K_TANH).astype(f32),
             Rd2[:, r0:r0 + ROWS_CORE], fcpad, Qc, Sd2], axis=1)
        in_maps.append({
            "blob": np.ascontiguousarray(blob),
            "redg": redg,
        })
    return in_maps


def kernel(contour: np.ndarray) -> np.ndarray:
    from concourse import bass_utils

    contour = np.asarray(contour, dtype=np.float32)
    if "nc" not in _compiled:
        _compiled["nc"] = _build()
    in_maps = _host_inputs(contour)
    res = bass_utils.run_bass_kernel_spmd(
        _compiled["nc"], in_maps, core_ids=list(range(N_CORES))).results

    mask = np.zeros((1, B, SIZE, SIZE), dtype=np.float32)
    for c in range(N_CORES):
        b = c // (N_CORES // B)
        r0 = (c % (N_CORES // B)) * ROWS_CORE
        mask[0, b, r0:r0 + ROWS_CORE, :] = (
            res[c]["out"].reshape(ROWS_CORE, SIZE))
    return mask
